# revision 44
# baseline (speedup 1.0000x reference)
"""Trainium2 Bass kernel for the dual-branch agent-attention module.

Sharding: data-parallel over B=8 (one batch element per NeuronCore).

Everything that depends only on (agent_input, weights) is computed on
the host and uploaded as per-batch constants:
  - Effective score weights Weff_A = Wq @ blockdiag(k12) and
    Weff_B = Wkhf @ blockdiag(qa12)  (associativity: the big
    activations never materialize q or kh at all).
  - Branch-A's per-agent exp bias c_A = blockdiag(k12)^T @ bq.
Math simplifications vs the reference:
  - Scalar softmax biases ba/bb cancel (softmax shift invariance).
  - v bias bv folds in after the xs softmax-normalize
    (xs_n = xs0/denom + bv); softmax denominators come from ones
    columns memset into the v tile.
  - proj bias is added host-side; the head-major permutation and the
    wa/wb/sqrt(D) score scales are folded into the uploaded weights.

Device dataflow (per core, N=4096 in 8 chunks of 512):
  stage 1 (per chunk): v = attnT^T@Wv, scores t = attnT^T@Weff_B
    (wide 512/256 rhs, k-major shared stationary operand), exp on ACT
    straight from PSUM, xs accumulated in PSUM across all 32 seq
    tiles (a single K=1 zeroing matmul opens the accumulation region;
    v matmuls run one chunk ahead).
  stage 1.5: xs normalize -> block-diag [xs | 1] tiles.
  stage 2 (per chunk): branch-A score matmuls + exp(+c_A) into pa,
    then the previous chunk's x_out/proj tiles (x_out = PA^T @ xs_bd
    with ones-column denominators, normalize, PE-transpose, proj,
    store bf16). The long per-tile serial chains hide under the next
    chunk's dense score streams.
"""

import os
import sys
import numpy as np

for _p in ("/opt/trn_rl_repo", os.path.expanduser("~/.axon_site/_ro/trn_rl_repo")):
    if os.path.isdir(_p) and _p not in sys.path:
        sys.path.insert(0, _p)

import ml_dtypes

import concourse.bass as bass
import concourse.bacc as bacc
import concourse.tile as tile
from concourse import mybir
from concourse.bass_utils import run_bass_kernel_spmd
from concourse.masks import make_identity

BF16 = mybir.dt.bfloat16
F32 = mybir.dt.float32
NPBF16 = ml_dtypes.bfloat16

B, N, NA, H, D = 8, 4096, 64, 12, 32
C = H * D            # 384
C2 = 2 * C           # 768
NP = H // 2          # 6 head pairs
CH = 512             # seq chunk
NCH = N // CH        # 8
TPC = CH // 128      # 4 seq tiles per chunk
SCALE = D ** -0.5

_CACHE = {}


def _build_bass(finalize=True, zero_bias=False):
    nc = bacc.Bacc()

    # ---- DRAM I/O ----
    xT = nc.dram_tensor("xT", [C, N], BF16, kind="ExternalInput")
    attnT = nc.dram_tensor("attnT", [C, N], BF16, kind="ExternalInput")
    weffa = nc.dram_tensor("weffa", [C, C2], BF16, kind="ExternalInput")
    weffb = nc.dram_tensor("weffb", [C, C2], BF16, kind="ExternalInput")
    wv = nc.dram_tensor("wv", [C, C], BF16, kind="ExternalInput")
    wproj = nc.dram_tensor("wproj", [C, C], BF16, kind="ExternalInput")
    if not zero_bias:
        cbav = nc.dram_tensor("cbav", [C2], F32, kind="ExternalInput")
        bvh = nc.dram_tensor("bvh", [2 * NP * D], F32, kind="ExternalInput")
    out = nc.dram_tensor("out", [N, C], BF16, kind="ExternalOutput")

    Exp = mybir.ActivationFunctionType.Exp

    with tile.TileContext(nc) as tc:
        with (
            tc.tile_pool(name="const", bufs=1) as const,
            tc.tile_pool(name="vv", bufs=2) as p_v,
            tc.tile_pool(name="pt", bufs=3) as p_pt,
            tc.tile_pool(name="xon", bufs=3) as p_xon,
            tc.tile_pool(name="xot", bufs=4) as p_xot,
            tc.tile_pool(name="osb", bufs=4) as p_out,
            tc.tile_pool(name="sm", bufs=4) as p_sm,
            tc.tile_pool(name="psA", bufs=3, space="PSUM") as psA,
            tc.tile_pool(name="psC", bufs=2, space="PSUM") as psC,
            tc.tile_pool(name="psT", bufs=2, space="PSUM") as psT,
            tc.tile_pool(name="psX", bufs=1, space="PSUM") as psX,
            # psA: 3 banks (B-512 / AC-scores / pr), psC: 2 (B-256 / xo),
            # psT: 2 (v / transposes), psX: 1 (xs acc) -> 8 banks.
        ):
            # ---- constants: wv + weffb first so stage-1 B starts ASAP ----
            w_v = const.tile([128, 3, C], BF16)
            w_eb = const.tile([128, 3, C2], BF16)
            w_ea = const.tile([128, 3, C2], BF16)
            w_pr = const.tile([128, 3, C], BF16)
            at_full = const.tile([128, 3, N], BF16)
            xt_full = const.tile([128, 3, N], BF16)
            at_r = attnT.rearrange("(k p) s -> p k s", p=128)
            xt_r = xT.rearrange("(k p) s -> p k s", p=128)

            # weffb + at0 + wv first on the sync queue: the score matmuls
            # (the PE's first dense stream, emitted before the v matmuls)
            # start as soon as these transfers land.
            nc.sync.dma_start(out=w_eb, in_=weffb.rearrange("(k p) m -> p k m", p=128))
            nc.sync.dma_start(out=at_full[:, :, 0:CH], in_=at_r[:, :, 0:CH])
            nc.sync.dma_start(out=w_v, in_=wv.rearrange("(k p) m -> p k m", p=128))
            nc.sync.dma_start(out=w_ea, in_=weffa.rearrange("(k p) m -> p k m", p=128))
            nc.sync.dma_start(out=w_pr, in_=wproj.rearrange("(k p) m -> p k m", p=128))
            for c in range(1, NCH):
                nc.sync.dma_start(out=at_full[:, :, c * CH:(c + 1) * CH],
                                  in_=at_r[:, :, c * CH:(c + 1) * CH])
            for c in range(NCH):
                nc.sync.dma_start(out=xt_full[:, :, c * CH:(c + 1) * CH],
                                  in_=xt_r[:, :, c * CH:(c + 1) * CH])

            cba = None
            if not zero_bias:
                cba = const.tile([128, 6], F32)
                nc.gpsimd.dma_start(out=cba, in_=cbav.rearrange("(j p) -> p j", p=128))
                bvb = const.tile([128, NP, D], F32)
                nc.gpsimd.dma_start(
                    out=bvb[0:64],
                    in_=bass.AP(tensor=bvh[:].tensor, offset=0,
                                ap=[[0, 64], [1, NP * D]]))
                nc.gpsimd.dma_start(
                    out=bvb[64:128],
                    in_=bass.AP(tensor=bvh[:].tensor, offset=NP * D,
                                ap=[[0, 64], [1, NP * D]]))
                touch = const.tile([128, 4], F32)
                nc.vector.tensor_copy(touch[:, 0:1], cba[:, 0:1])
                nc.vector.tensor_copy(touch[:, 1:2], bvb[:, 0:1, 0])
            ident = const.tile([128, 128], BF16)
            make_identity(nc, ident)
            zrow = const.tile([1, 396], BF16)
            nc.vector.memset(zrow, 0.0)
            pa_full = const.tile([128, 6, N], BF16)

            # ---- xs accumulator: open the PSUM region with a zero matmul ----
            xs_acc = psX.tile([128, 6, 66], F32)
            nc.tensor.matmul(xs_acc[:, :, :], lhsT=zrow[:, 0:128], rhs=zrow[:, 0:396],
                             start=True, stop=False, skip_group_check=True)

            # ---- stage 1: values + branch-B attention (xs in PSUM) ----
            # v matmuls are interleaved per tile behind the score streams
            # (chunk 0's own during chunk 0, then chunk c+1's during chunk
            # c), so the psT slots rotate cleanly and PE never bursts v
            # matmuls back-to-back against a slow consumer.
            v_tiles = {}

            def alloc_v(c):
                v_t = p_v.tile([128, TPC, H, 33], BF16)
                nc.vector.memset(v_t[:, :, :, 32], 1.0)
                v_tiles[c] = v_t

            def emit_v_tile(c, t):
                s0 = c * CH + t * 128
                psv = psT.tile([128, C], F32, tag="pT")
                for k in range(3):
                    nc.tensor.matmul(psv, lhsT=at_full[:, k, s0:s0 + 128],
                                     rhs=w_v[:, k, :],
                                     start=(k == 0), stop=(k == 2))
                nc.vector.tensor_copy(
                    v_tiles[c][:, t, :, 0:32],
                    psv[:].rearrange("p (h d) -> p h d", d=32))

            alloc_v(0)
            pending_xs = None
            for c in range(NCH):
                v_t = v_tiles[c]
                for t in range(TPC):
                    s0 = c * CH + t * 128
                    ps4 = psA.tile([128, 512], F32, tag="pA")
                    ps2 = psC.tile([128, 256], F32, tag="pC")
                    for k in range(3):
                        at_k = at_full[:, k, s0:s0 + 128]
                        nc.tensor.matmul(ps4, lhsT=at_k, rhs=w_eb[:, k, 0:512],
                                         start=(k == 0), stop=(k == 2))
                        nc.tensor.matmul(ps2, lhsT=at_k, rhs=w_eb[:, k, 512:768],
                                         start=(k == 0), stop=(k == 2))
                    if c == 0:
                        emit_v_tile(0, t)
                    pt = p_pt.tile([128, 768], BF16)
                    nc.scalar.activation(pt[:, 0:512], ps4, Exp)
                    nc.scalar.activation(pt[:, 512:768], ps2, Exp)
                    if pending_xs is not None:
                        pending_xs()
                    if c + 1 < NCH:
                        if t == 0:
                            alloc_v(c + 1)
                        emit_v_tile(c + 1, t)
                    last = (c == NCH - 1 and t == TPC - 1)

                    def make_xs(pt=pt, v_t=v_t, t=t, last=last):
                        def emit():
                            for j in range(6):
                                nc.tensor.matmul(
                                    xs_acc[:, j, :], lhsT=pt[:, j * 128:(j + 1) * 128],
                                    rhs=v_t[:, t, 2 * j:2 * j + 2, :],
                                    start=False, stop=(last and j == 5),
                                    skip_group_check=True)
                        return emit
                    pending_xs = make_xs()
            pending_xs()

            # ---- stage 1.5: xs normalize -> block-diag [xs | 1] tiles ----
            xs_bd = const.tile([128, 6, 66], BF16)
            nc.vector.memset(xs_bd, 0.0)
            nc.vector.memset(xs_bd[0:64, :, 32:33], 1.0)
            nc.vector.memset(xs_bd[64:128, :, 65:66], 1.0)
            rec6 = p_sm.tile([128, 6], F32, tag="rec")
            nc.vector.reciprocal(rec6[0:64, :], xs_acc[0:64, :, 32])
            nc.vector.reciprocal(rec6[64:128, :], xs_acc[64:128, :, 65])
            nc.vector.tensor_mul(xs_bd[0:64, :, 0:32], xs_acc[0:64, :, 0:32],
                                 rec6[0:64, :].unsqueeze(2).to_broadcast([64, 6, 32]))
            nc.vector.tensor_mul(xs_bd[64:128, :, 33:65], xs_acc[64:128, :, 33:65],
                                 rec6[64:128, :].unsqueeze(2).to_broadcast([64, 6, 32]))
            if not zero_bias:
                nc.vector.tensor_add(xs_bd[0:64, :, 0:32], xs_bd[0:64, :, 0:32],
                                     bvb[0:64])
                nc.vector.tensor_add(xs_bd[64:128, :, 33:65], xs_bd[64:128, :, 33:65],
                                     bvb[64:128])

            # ---- stage 2: branch-A scores + attention + proj ----
            def stage2_tiles(c, ts):
                for t in ts:
                    s0 = c * CH + t * 128
                    xo = psC.tile([128, 396], F32, tag="pC")
                    for j in range(6):
                        nc.tensor.matmul(xo[:, j * 66:(j + 1) * 66],
                                         lhsT=pa_full[:, j, s0:s0 + 128],
                                         rhs=xs_bd[:, j, :],
                                         start=True, stop=True)
                    xo3 = xo[:].rearrange("p (h d) -> p h d", d=33)
                    rec = p_sm.tile([128, 12], F32, tag="rec12")
                    nc.vector.reciprocal(rec, xo3[:, :, 32])
                    xon = p_xon.tile([128, C], BF16)
                    nc.vector.tensor_mul(xon[:].rearrange("p (h d) -> p h d", d=32),
                                         xo3[:, :, 0:32],
                                         rec[:].unsqueeze(2).to_broadcast([128, 12, 32]))
                    pr = psA.tile([128, C], F32, tag="pA")
                    for f in range(3):
                        tp = psT.tile([128, 128], BF16, tag="pT")
                        nc.tensor.transpose(tp, xon[:, f * 128:(f + 1) * 128], ident)
                        xot = p_xot.tile([128, 128], BF16)
                        nc.vector.tensor_copy(xot, tp)
                        nc.tensor.matmul(pr, lhsT=xot, rhs=w_pr[:, f, :],
                                         start=(f == 0), stop=(f == 2),
                                         skip_group_check=True)
                    o_sb = p_out.tile([128, C], BF16)
                    if t % 2 == 0:
                        nc.scalar.copy(o_sb, pr)
                    else:
                        nc.vector.tensor_copy(o_sb, pr)
                    nc.sync.dma_start(out=out[s0:s0 + 128, :], in_=o_sb)

            for c in range(NCH):
                for j in range(6):
                    ps = psA.tile([128, CH], F32, tag="pA")
                    for k in range(3):
                        nc.tensor.matmul(ps, lhsT=w_ea[:, k, j * 128:(j + 1) * 128],
                                         rhs=xt_full[:, k, c * CH:(c + 1) * CH],
                                         start=(k == 0), stop=(k == 2))
                    nc.scalar.activation(
                        pa_full[:, j, c * CH:(c + 1) * CH], ps, Exp,
                        bias=(0.0 if zero_bias else cba[:, j:j + 1]))
                    if c > 0 and j == 2:
                        stage2_tiles(c - 1, (0, 1))
                if c > 0:
                    stage2_tiles(c - 1, (2, 3))
            stage2_tiles(NCH - 1, (0, 1, 2, 3))
    if finalize:
        nc.finalize()
    return nc


def _prep_host(inputs):
    f32 = np.float32
    x = np.asarray(inputs["x"], f32)
    attn = np.asarray(inputs["attn"], f32)
    agent = np.asarray(inputs["agent_input"], f32)
    wa = np.asarray(inputs["wa"], f32)
    wb = np.asarray(inputs["wb"], f32)

    # head-major permutation: (h, br, d) -> h*64 + br*32 + d, with the
    # branch score scales (wa/wb * D^-0.5) folded into the k-side weights
    perm = np.empty(C2, np.int64)
    sva = np.empty(C2, f32)
    svb = np.empty(C2, f32)
    for h in range(H):
        for br in range(2):
            j0 = h * 64 + br * 32
            perm[j0:j0 + 32] = br * C + h * 32 + np.arange(32)
            sva[j0:j0 + 32] = wa[br] * SCALE
            svb[j0:j0 + 32] = wb[br] * SCALE

    wq_p = np.asarray(inputs["Wq_lf"], f32)[:, perm]
    bq_p = np.asarray(inputs["bq_lf"], f32)[perm]
    wkag_p = np.asarray(inputs["Wk_ag"], f32)[:, perm] * sva[None, :]
    bkag_p = np.asarray(inputs["bk_ag"], f32)[perm] * sva
    wqag_p = np.asarray(inputs["Wq_ag"], f32)[:, perm]
    bqag_p = np.asarray(inputs["bq_ag"], f32)[perm]
    wkhf_p = np.asarray(inputs["Wk_hf"], f32)[:, perm] * svb[None, :]

    zb = all(not np.any(np.asarray(inputs[k]))
             for k in ("bq_lf", "bk_ag", "bq_ag", "bk_hf", "bv_hf", "ba", "bb"))

    shared = {
        "wv": np.asarray(inputs["Wv_hf"], f32).astype(NPBF16),
        "wproj": np.asarray(inputs["Wproj"], f32).astype(NPBF16),
    }
    if not zb:
        bv_in = np.asarray(inputs["bv_hf"], f32)
        # bvh[half, j, d]: half 0 = head 2j, half 1 = head 2j+1
        bvh = np.empty((2, NP, D), f32)
        for j in range(NP):
            bvh[0, j, :] = bv_in[(2 * j) * D:(2 * j + 1) * D]
            bvh[1, j, :] = bv_in[(2 * j + 1) * D:(2 * j + 2) * D]
        shared["bvh"] = np.ascontiguousarray(bvh.reshape(-1))

    # per-batch: agent projections -> block-diag -> effective weights
    kag = agent @ wkag_p + bkag_p          # [B, 64, 768]
    qa = agent @ wqag_p + bqag_p           # [B, 64, 768]
    weffa = np.zeros((B, C, C2), f32)
    weffb = np.zeros((B, C, C2), f32)
    cba = np.zeros((B, C2), f32)
    for j in range(NP):
        j0 = j * 128
        for half, hlo in ((slice(j0, j0 + 64), slice(0, 64)),
                          (slice(j0 + 64, j0 + 128), slice(64, 128))):
            # k12_j block: [projdims 64, agents 64] = kag[:, :, half].T
            kj = kag[:, :, half].transpose(0, 2, 1)      # [B, 64p, 64a]
            qj = qa[:, :, half].transpose(0, 2, 1)
            weffa[:, :, j0:j0 + 128][:, :, hlo] = wq_p[:, half] @ kj
            weffb[:, :, j0:j0 + 128][:, :, hlo] = wkhf_p[:, half] @ qj
            cba[:, j0:j0 + 128][:, hlo] = np.einsum(
                "bpa,p->ba", kj, bq_p[half])

    xT = np.ascontiguousarray(x.transpose(0, 2, 1)).astype(NPBF16)
    attnT = np.ascontiguousarray(attn.transpose(0, 2, 1)).astype(NPBF16)
    weffa_b = weffa.astype(NPBF16)
    weffb_b = weffb.astype(NPBF16)
    in_maps = []
    for b in range(B):
        m = dict(shared)
        m["xT"] = xT[b]
        m["attnT"] = attnT[b]
        m["weffa"] = weffa_b[b]
        m["weffb"] = weffb_b[b]
        if not zb:
            # cbav flat layout: value for (pair j, agent-col a) at j*128+a
            m["cbav"] = np.ascontiguousarray(cba[b])
        in_maps.append(m)
    return in_maps, zb


def kernel(**inputs):
    in_maps, zb = _prep_host(inputs)
    key = ("nc", zb)
    if key not in _CACHE:
        _CACHE[key] = _build_bass(zero_bias=zb)
    nc = _CACHE[key]
    res = run_bass_kernel_spmd(nc, in_maps, core_ids=list(range(B)))
    outs = np.stack([np.asarray(res.results[b]["out"], np.float32)
                     for b in range(B)], axis=0)
    if not zb:
        outs = outs + np.asarray(inputs["bproj"], np.float32)[None, None, :]
    return outs
